# revision 1
# baseline (speedup 1.0000x reference)
"""Trainium2 Bass kernel for nn_AttackLoss (nms_detection).

Computes, for O=2048 ground-truth boxes vs D=8732 detections:
    best[o]  = max IoU over same-label detections of object o
    loss     = sum(has_match * (1 - best)) / sum(has_match)

Sharding (primary, "bucket" kernel): the label-equality mask makes the
[O, D] IoU matrix block-sparse — only same-class pairs matter. The host
sorts both sides by class and shards whole classes across the 8 cores
(S_SLOTS=3 class-slots per core, 21 classes). Each slot puts one class's
<=128 objects on SBUF partitions and its detections on the free axis
(rank-dependent capacity FDS=[472,432,408]),
so a core computes only its classes' dense blocks (~21x less work than the
full cross product). Per slot the DVE runs 5-6 fused passes (custom DVE
ops: wx/wy = relu(min-max), t3 = area_o - wx*wy, denom add, approximate
reciprocal, iou with a fused running-max accumulator); detection rows are
broadcast across partitions by GpSimd partition_broadcast and by DMA
stride-0 reads so the DVE only does IoU math. Each core ships its raw
[128, 2] accumulator (per-partition sums of matched (1-best) and match
count); the host folds partitions and cores in one gather and divides.

Fallback ("dense" kernel, any input): objects sharded 256/core on the free
axis, all dets on partitions, label mask applied explicitly. Used when a
class exceeds the bucket kernel's static capacity (FDS[rank] dets or 128
objects per class, or more than 24 classes).
"""

from contextlib import ExitStack

import numpy as np

import concourse.bacc as bacc
import concourse.bass as bass
import concourse.mybir as mybir
import concourse.tile as tile
from concourse.bass_isa import ReduceOp
from concourse.bass_utils import run_bass_kernel_spmd

F32 = mybir.dt.float32
OP = mybir.AluOpType
AX = mybir.AxisListType

N_CORES = 8
N_DET = 8732
N_OBJ = 2048
N_CLASSES = 21
OBJ_PER_CORE = N_OBJ // N_CORES  # 256
T_DET = 69                        # ceil(8732/128)
DET_PAD = 128 * T_DET             # 8832

# bucketed kernel static capacity: slot s holds a core's rank-s class
# (classes sorted by det count, descending), so later slots can be smaller.
S_SLOTS = 3      # class-slots per core
FDS = [472, 432, 408]   # det capacity per slot rank
FD = FDS[0]
FD_OFF = [sum(5 * f for f in FDS[:s]) for s in range(S_SLOTS)]  # row offsets
FD_TOT = sum(FDS)
MAX_SLOTS = N_CORES * S_SLOTS


# ---------------------------------------------------------------------------
# custom DVE ops

_OPS_REGISTERED = {}


def _register_custom_ops():
    """Register fused DVE ops (official extension point: dve_ops.OPS)."""
    if _OPS_REGISTERED:
        return _OPS_REGISTERED
    import concourse.dve_ops as dve_ops
    from concourse.dve_spec import (Spec, Src0, Src1, C0, C1, relu, maxx,
                                    minn, lower)
    from concourse.dve_uop import DveOpSpec

    def make(name, spec, subdim=False):
        if name in dve_ops._SUB_OPCODE_FOR_NAME:
            for op in dve_ops.OPS:
                if op.name == name:
                    return op
        row = dve_ops._CUSTOM_DVE_ROW_BASE + len(dve_ops.OPS)
        assert row < 0x20
        shas = {}
        from concourse.dve_spec import _has_src1
        for ver in ("v3", "v4"):
            uops = lower(spec, ver=ver)
            shas[ver] = DveOpSpec(name=name, opcode=row, uops=uops,
                                  rd1_en=_has_src1(spec)).sha(ver)
        op = dve_ops.DveOp(name, spec, subdim, shas)
        dve_ops.OPS.append(op)
        dve_ops.CUSTOM_DVE_SPECS[name] = spec
        dve_ops._SUB_OPCODE_FOR_NAME[name] = row
        return op

    def _wx_ref(in0, in1, s0, s1, imm2):
        return np.maximum(
            np.minimum(in0.astype(np.float32), s0)
            - np.maximum(in1.astype(np.float32), s1), 0.0)

    # wx = relu(min(d_hi, o_hi) - max(d_lo, o_lo))
    wx_op = make("IOU_WX_ANT", Spec(
        body=relu(minn(Src0, C0) - maxx(Src1, C1)),
        reference=_wx_ref,
    ))

    def _t3_ref(in0, in1, s0, s1, imm2):
        return (s0 - in0.astype(np.float32) * in1).astype(np.float32)

    # t3 = area_o - wx*wy  (= area_o - inter)
    t3_op = make("IOU_T3_ANT", Spec(
        body=C0 - Src0 * Src1,
        reference=_t3_ref,
    ))

    def _ioumax_ref(in0, in1, s0, s1, imm2):
        b = ((s0 - in0.astype(np.float32)) * in1).astype(np.float32)
        b2 = b.reshape(b.shape[0], -1)
        seed = np.asarray(s1, np.float32).reshape(-1, 1) if isinstance(
            s1, np.ndarray) else np.full((b2.shape[0], 1), s1, np.float32)
        return b, np.maximum(b2.max(axis=-1, keepdims=True), seed)

    # iou = (area_o - t3) * recip ; accum_out = max(iou) over free dim
    ioumax_op = make("IOU_MAX_ANT", Spec(
        body=(C0 - Src0) * Src1,
        accum=maxx,
        accum_init=C1,
        reference=_ioumax_ref,
    ))

    def _contrib_ref(in0, in1, s0, s1, imm2):
        return (in0.astype(np.float32)
                + (1.0 - in1.astype(np.float32)) * s0).astype(np.float32)

    # acc += (1 - best) * veff
    from concourse.dve_spec import One
    contrib_op = make("IOU_CONTRIB_ANT", Spec(
        body=Src0 + (One - Src1) * C0,
        reference=_contrib_ref,
    ))

    _OPS_REGISTERED.update(wx=wx_op, t3=t3_op, ioumax=ioumax_op,
                           contrib=contrib_op)
    return _OPS_REGISTERED


def _build_dense():
    """Dense kernel: all dets (on partitions) x this core's objects (free)."""
    nc = bacc.Bacc("TRN2", target_bir_lowering=False, debug=False,
                   num_devices=N_CORES)
    F = OBJ_PER_CORE

    detp_d = nc.dram_tensor("detp", [128, 5, T_DET], F32, kind="ExternalInput")
    objr_d = nc.dram_tensor("objr", [5, F], F32, kind="ExternalInput")
    part_d = nc.dram_tensor("partial", [1, 2], F32, kind="ExternalOutput")

    with tile.TileContext(nc) as tc, ExitStack() as ctx:
        cpool = ctx.enter_context(tc.tile_pool(name="const", bufs=1))
        wpool = ctx.enter_context(tc.tile_pool(name="work", bufs=3))

        detp = cpool.tile([128, 5, T_DET], F32)
        nc.sync.dma_start(detp[:], detp_d[:])
        # broadcast object rows across partitions
        names = ["ox1", "oy1", "ox2", "oy2", "olab"]
        ob = {}
        for i, nm in enumerate(names):
            row = cpool.tile([1, F], F32, tag=f"r_{nm}")
            nc.sync.dma_start(row[:], objr_d[i:i + 1, :])
            t = cpool.tile([128, F], F32, tag=f"b_{nm}")
            nc.gpsimd.partition_broadcast(t[:], row[:], channels=128)
            ob[nm] = t

        # object areas [128, F]
        aob = cpool.tile([128, F], F32)
        wob = wpool.tile([128, F], F32, tag="wob")
        nc.vector.tensor_tensor(wob[:], ob["ox2"][:], ob["ox1"][:], OP.subtract)
        hob = wpool.tile([128, F], F32, tag="hob")
        nc.vector.tensor_tensor(hob[:], ob["oy2"][:], ob["oy1"][:], OP.subtract)
        nc.vector.tensor_tensor(aob[:], wob[:], hob[:], OP.mult)

        # det areas [128, T]
        ad = cpool.tile([128, T_DET], F32)
        wd = wpool.tile([128, T_DET], F32, tag="wd")
        nc.vector.tensor_tensor(wd[:], detp[:, 2, :], detp[:, 0, :], OP.subtract)
        hd = wpool.tile([128, T_DET], F32, tag="hd")
        nc.vector.tensor_tensor(hd[:], detp[:, 3, :], detp[:, 1, :], OP.subtract)
        nc.vector.tensor_tensor(ad[:], wd[:], hd[:], OP.mult)

        bmax = cpool.tile([128, F], F32)
        nc.vector.memset(bmax[:], 0.0)
        hm = cpool.tile([128, F], F32)
        nc.vector.memset(hm[:], 0.0)

        for t in range(T_DET):
            dx1 = detp[:, 0, t:t + 1]
            dy1 = detp[:, 1, t:t + 1]
            dx2 = detp[:, 2, t:t + 1]
            dy2 = detp[:, 3, t:t + 1]
            dlab = detp[:, 4, t:t + 1]
            adt = ad[:, t:t + 1]

            mnx = wpool.tile([128, F], F32, tag="mnx")
            nc.vector.tensor_scalar(mnx[:], ob["ox2"][:], dx2, None, op0=OP.min)
            mxx = wpool.tile([128, F], F32, tag="mxx")
            nc.vector.tensor_scalar(mxx[:], ob["ox1"][:], dx1, None, op0=OP.max)
            wx = wpool.tile([128, F], F32, tag="wx")
            nc.vector.tensor_tensor(wx[:], mnx[:], mxx[:], OP.subtract)
            wxr = wpool.tile([128, F], F32, tag="wxr")
            nc.vector.tensor_scalar(wxr[:], wx[:], 0.0, None, op0=OP.max)

            mny = wpool.tile([128, F], F32, tag="mny")
            nc.vector.tensor_scalar(mny[:], ob["oy2"][:], dy2, None, op0=OP.min)
            mxy = wpool.tile([128, F], F32, tag="mxy")
            nc.vector.tensor_scalar(mxy[:], ob["oy1"][:], dy1, None, op0=OP.max)
            wy = wpool.tile([128, F], F32, tag="wy")
            nc.vector.tensor_tensor(wy[:], mny[:], mxy[:], OP.subtract)
            wyr = wpool.tile([128, F], F32, tag="wyr")
            nc.vector.tensor_scalar(wyr[:], wy[:], 0.0, None, op0=OP.max)

            inter = wpool.tile([128, F], F32, tag="inter")
            nc.vector.tensor_tensor(inter[:], wxr[:], wyr[:], OP.mult)
            sab = wpool.tile([128, F], F32, tag="sab")
            nc.vector.tensor_scalar(sab[:], aob[:], adt, None, op0=OP.add)
            denom = wpool.tile([128, F], F32, tag="denom")
            nc.vector.tensor_tensor(denom[:], sab[:], inter[:], OP.subtract)
            rec = wpool.tile([128, F], F32, tag="rec")
            nc.vector.reciprocal(rec[:], denom[:])
            iou = wpool.tile([128, F], F32, tag="iou")
            nc.vector.tensor_tensor(iou[:], inter[:], rec[:], OP.mult)

            eq = wpool.tile([128, F], F32, tag="eq")
            nc.vector.tensor_scalar(eq[:], ob["olab"][:], dlab, None,
                                    op0=OP.is_equal)
            miou = wpool.tile([128, F], F32, tag="miou")
            nc.vector.tensor_tensor(miou[:], iou[:], eq[:], OP.mult)

            nc.vector.tensor_tensor(bmax[:], bmax[:], miou[:], OP.max)
            nc.vector.tensor_tensor(hm[:], hm[:], eq[:], OP.max)

        bred = cpool.tile([128, F], F32)
        nc.gpsimd.partition_all_reduce(bred[:], bmax[:], 128, ReduceOp.max)
        hred = cpool.tile([128, F], F32)
        nc.gpsimd.partition_all_reduce(hred[:], hm[:], 128, ReduceOp.max)

        c1 = wpool.tile([1, F], F32, tag="c1")
        nc.vector.tensor_scalar(c1[:], bred[0:1, :], -1.0, 1.0,
                                op0=OP.mult, op1=OP.add)
        c2 = wpool.tile([1, F], F32, tag="c2")
        nc.vector.tensor_tensor(c2[:], c1[:], hred[0:1, :], OP.mult)

        outt = wpool.tile([1, 2], F32, tag="outt")
        nc.vector.tensor_reduce(outt[:, 0:1], c2[:], AX.X, OP.add)
        nc.vector.tensor_reduce(outt[:, 1:2], hred[0:1, :], AX.X, OP.add)
        nc.sync.dma_start(part_d[:], outt[:])

    nc.compile()
    return nc


def _build_bucket(fast_recip=True, nr_refine=False, repeat=1):
    """Class-bucketed kernel: each core runs S_SLOTS single-class slots.

    Slot s is (<=128 objects of one class on partitions) x (<=FDS[s] dets of
    the same class on the free axis); no label masking is needed inside a
    slot. Det rows (x2, x1, y2, y1, area per slot) are broadcast across
    partitions by GpSimd partition_broadcast and by DMAs with stride-0
    partition reads from DRAM, keeping the DVE free for the IoU math.
    """
    ops = _register_custom_ops()
    from concourse.dve_ops import (RECIPROCAL_APPROX_FAST,
                                   RECIPROCAL_APPROX_NR,
                                   RECIP_APPROX_FAST_CONSTS)

    nc = bacc.Bacc("TRN2", target_bir_lowering=False, debug=False,
                   num_devices=N_CORES)

    detrow_d = nc.dram_tensor("detrow", [1, 5 * FD_TOT], F32,
                              kind="ExternalInput")
    objs_d = nc.dram_tensor("objs", [128, S_SLOTS, 6], F32,
                            kind="ExternalInput")
    part_d = nc.dram_tensor("partial", [128, 2], F32, kind="ExternalOutput")

    def drow(t, s, k0, k1):
        f = FDS[s]
        return t[:, FD_OFF[s] + k0 * f:FD_OFF[s] + k1 * f]

    with tile.TileContext(nc) as tc, ExitStack() as ctx:
        cpool = ctx.enter_context(tc.tile_pool(name="const", bufs=1))
        wpool = ctx.enter_context(tc.tile_pool(name="work", bufs=2))
        bpool = ctx.enter_context(tc.tile_pool(name="bcast", bufs=S_SLOTS))

        # slot-0's x rows gate the first DVE op: fetch them with a DMA
        # broadcast straight from DRAM; the staging row tensor and the
        # object slab ride the other queues.
        detrow = cpool.tile([1, 5 * FD_TOT], F32)
        nc.sync.dma_start(detrow[:], detrow_d[:])
        objs = cpool.tile([128, S_SLOTS, 6], F32)
        nc.gpsimd.dma_start(objs[:], objs_d[:])
        bcx0 = bpool.tile([128, 2 * FDS[0]], F32, tag="bcx")
        srcx, _ = bass.broadcast_tensor_aps(drow(detrow_d, 0, 0, 2), bcx0[:])
        nc.scalar.dma_start(bcx0[:], srcx)

        acc = cpool.tile([128, 2], F32)
        nc.vector.memset(acc[:], 0.0)

        for rep in range(repeat):
          bxs = [bcx0]
          if rep > 0:
            bc0r = bpool.tile([128, 2 * FDS[0]], F32, tag="bcx",
                              name=f"bcx0_{rep}")
            srcx, _ = bass.broadcast_tensor_aps(drow(detrow_d, 0, 0, 2),
                                                bc0r[:])
            nc.sync.dma_start(bc0r[:], srcx)
            bxs = [bc0r]
          bys = []
          for s in range(1, S_SLOTS):
            bxs.append(bpool.tile([128, 2 * FDS[s]], F32, tag="bcx",
                                  name=f"bcx{s}_{rep}"))
          for s in range(S_SLOTS):
            bys.append(bpool.tile([128, 2 * FDS[s]], F32, tag="bcy",
                                  name=f"bcy{s}_{rep}"))

          for s in range(S_SLOTS):
            f = FDS[s]
            bcx, bcy = bxs[s], bys[s]
            if s == S_SLOTS - 1:
                nc.gpsimd.partition_broadcast(
                    bcx[:], drow(detrow, s, 0, 2), channels=128)
                srcy, _ = bass.broadcast_tensor_aps(
                    drow(detrow_d, s, 2, 4), bcy[:])
                nc.sync.dma_start(bcy[:], srcy)
            else:
                if s > 0:
                    nc.gpsimd.partition_broadcast(
                        bcx[:], drow(detrow, s, 0, 2), channels=128)
                nc.gpsimd.partition_broadcast(
                    bcy[:], drow(detrow, s, 2, 4), channels=128)
            # area row: DMA broadcast from DRAM for every slot
            adt = wpool.tile([128, FD], F32, tag="adt")
            srca, _ = bass.broadcast_tensor_aps(drow(detrow_d, s, 4, 5),
                                                adt[:, 0:f])
            nc.sync.dma_start(adt[:, 0:f], srca)

            dx2b = bcx[:, 0:f]
            dx1b = bcx[:, f:2 * f]
            dy2b = bcy[:, 0:f]
            dy1b = bcy[:, f:2 * f]
            adb = adt[:, 0:f]

            ox1 = objs[:, s, 0:1]
            oy1 = objs[:, s, 1:2]
            ox2 = objs[:, s, 2:3]
            oy2 = objs[:, s, 3:4]
            ao = objs[:, s, 4:5]
            veff = objs[:, s, 5:6]

            wx = wpool.tile([128, FD], F32, tag="wx")
            wy = wpool.tile([128, FD], F32, tag="wy")
            # slot 0's y rows (GpSimd broadcast) land before its x rows
            # (DMA + 900ns sem): emit WY first so it doesn't queue behind
            # WX on the in-order DVE
            halves = [("wy", wy, dy2b, dy1b, oy2, oy1),
                      ("wx", wx, dx2b, dx1b, ox2, ox1)]
            if s != 0:
                halves.reverse()
            winsts = []
            for _, wt, dhi, dlo, ohi, olo in halves:
                winsts.append(
                    nc.vector._custom_dve(ops["wx"], out=wt[:, 0:f], in0=dhi,
                                          in1=dlo, s0=ohi, s1=olo))
            if s == 0:
                # slot-0's y rows (GpSimd bcast) land before its x rows
                # (DMA + 900ns sem): pin WY ahead of WX on the in-order DVE
                from concourse.tile_rust import add_dep_helper
                add_dep_helper(winsts[1].ins, winsts[0].ins, sync=False,
                               reason="WY before WX: y data lands first")
            t3 = wpool.tile([128, FD], F32, tag="t3")
            nc.vector._custom_dve(ops["t3"], out=t3[:, 0:f], in0=wx[:, 0:f],
                                  in1=wy[:, 0:f], s0=ao)
            denom = wpool.tile([128, FD], F32, tag="denom")
            # slot 1's add runs on GpSimd: mid-kernel the DVE is saturated
            # and Pool has slack there (slot 2's would stall the DVE tail)
            (nc.gpsimd if s == 1 else nc.vector).tensor_tensor(
                denom[:, 0:f], t3[:, 0:f], adb, OP.add)
            rec = wpool.tile([128, FD], F32, tag="rec")
            if fast_recip and nr_refine:
                r0 = wpool.tile([128, FD], F32, tag="r0")
                nc.vector._custom_dve(RECIPROCAL_APPROX_FAST, out=r0[:, 0:f],
                                      in0=denom[:, 0:f],
                                      **RECIP_APPROX_FAST_CONSTS)
                nc.vector._custom_dve(RECIPROCAL_APPROX_NR, out=rec[:, 0:f],
                                      in0=denom[:, 0:f], in1=r0[:, 0:f],
                                      s0=2.0)
            elif fast_recip:
                nc.vector._custom_dve(RECIPROCAL_APPROX_FAST, out=rec[:, 0:f],
                                      in0=denom[:, 0:f],
                                      **RECIP_APPROX_FAST_CONSTS)
            else:
                nc.vector.reciprocal(rec[:, 0:f], denom[:, 0:f])

            scratch = wpool.tile([128, FD], F32, tag="scratch")
            best = wpool.tile([128, 1], F32, tag="best")
            nc.vector._custom_dve(ops["ioumax"], out=scratch[:, 0:f],
                                  accum_out=best[:], in0=t3[:, 0:f],
                                  in1=rec[:, 0:f], s0=ao, s1=0.0)

            # acc[:,0] += (1-best)*veff
            nc.vector._custom_dve(ops["contrib"], out=acc[:, 0:1],
                                  in0=acc[:, 0:1], in1=best[:], s0=veff)

        # npos: one reduce over the veff columns of all slots
        nc.vector.tensor_reduce(acc[:, 1:2], objs[:, :, 5], AX.X, OP.add)

        # ship the raw per-partition accumulator; the host folds the 128
        # rows during its cross-core gather (saves the Pool allreduce
        # round-trip on the kernel tail)
        nc.sync.dma_start(part_d[:], acc[:])

    nc.compile()
    return nc


def _prep_bucket_inputs(det_boxes, det_labels, boxes, labels):
    """Build per-core in_maps for the bucketed kernel, or None if the
    static capacity (S_SLOTS per core, FDS[rank] dets / 128 objects per
    class) doesn't fit this input."""
    det_boxes = det_boxes.astype(np.float32)
    boxes = boxes.astype(np.float32)
    det_labels = np.asarray(det_labels)
    labels = np.asarray(labels)

    if det_labels.min() < 0 or labels.min() < 0:
        return None
    ncls = int(max(N_CLASSES, det_labels.max() + 1, labels.max() + 1))
    dc = np.bincount(det_labels, minlength=ncls)
    oc = np.bincount(labels, minlength=ncls)
    if ncls > MAX_SLOTS or oc.max() > 128:
        return None
    # rank classes by det count (desc); rank r lands in slot r//N_CORES of
    # core r%N_CORES, so slot capacities can shrink with rank
    rank_order = np.argsort(-dc, kind="stable")
    for r, cls in enumerate(rank_order):
        if dc[cls] > FDS[r // N_CORES]:
            return None

    det_order = np.argsort(det_labels, kind="stable")
    obj_order = np.argsort(labels, kind="stable")
    det_off = np.concatenate([[0], np.cumsum(dc)])
    obj_off = np.concatenate([[0], np.cumsum(oc)])

    in_maps = []
    for c in range(N_CORES):
        detr = np.full(5 * FD_TOT, -5.0, dtype=np.float32)
        for s in range(S_SLOTS):
            f = FDS[s]
            detr[FD_OFF[s] + 4 * f:FD_OFF[s] + 5 * f] = 0.0  # pad area
        objs = np.zeros((128, S_SLOTS, 6), dtype=np.float32)
        objs[:, :, 0] = -9.0
        objs[:, :, 1] = -9.0
        objs[:, :, 2] = -8.0
        objs[:, :, 3] = -8.0
        objs[:, :, 4] = 1.0
        for s in range(S_SLOTS):
            r = s * N_CORES + c
            if r >= ncls:
                continue
            cls = rank_order[r]
            f = FDS[s]
            dsel = det_order[det_off[cls]:det_off[cls + 1]]
            osel = obj_order[obj_off[cls]:obj_off[cls + 1]]
            nd, no = len(dsel), len(osel)
            db = det_boxes[dsel]
            o = FD_OFF[s]
            detr[o + 0 * f:o + 0 * f + nd] = db[:, 2]
            detr[o + 1 * f:o + 1 * f + nd] = db[:, 0]
            detr[o + 2 * f:o + 2 * f + nd] = db[:, 3]
            detr[o + 3 * f:o + 3 * f + nd] = db[:, 1]
            detr[o + 4 * f:o + 4 * f + nd] = (
                (db[:, 2] - db[:, 0]) * (db[:, 3] - db[:, 1]))
            ob = boxes[osel]
            objs[:no, s, 0:4] = ob
            objs[:no, s, 4] = (ob[:, 2] - ob[:, 0]) * (ob[:, 3] - ob[:, 1])
            objs[:no, s, 5] = 1.0 if nd > 0 else 0.0
        in_maps.append({"detrow": detr.reshape(1, 5 * FD_TOT), "objs": objs})
    return in_maps


def _prep_dense_inputs(det_boxes, det_labels, boxes, labels):
    """Build per-core in_maps for the dense kernel (numpy only)."""
    det = np.full((DET_PAD, 5), -5.0, dtype=np.float32)
    det[:N_DET, 0:4] = det_boxes.astype(np.float32)
    det[:N_DET, 4] = det_labels.astype(np.float32)
    det[N_DET:, 4] = -1.0
    # [DET_PAD, 5] -> [T, 128, 5] -> [128, 5, T]
    detp = np.ascontiguousarray(
        det.reshape(T_DET, 128, 5).transpose(1, 2, 0))

    in_maps = []
    for c in range(N_CORES):
        sl = slice(c * OBJ_PER_CORE, (c + 1) * OBJ_PER_CORE)
        objr = np.empty((5, OBJ_PER_CORE), dtype=np.float32)
        objr[0:4, :] = boxes[sl].astype(np.float32).T
        objr[4, :] = labels[sl].astype(np.float32)
        in_maps.append({"detp": detp, "objr": objr})
    return in_maps


_CACHE = {}


def _get_dense():
    if "dense" not in _CACHE:
        _CACHE["dense"] = _build_dense()
    return _CACHE["dense"]


def _get_bucket():
    if "bucket" not in _CACHE:
        _CACHE["bucket"] = _build_bucket()
    return _CACHE["bucket"]


def _run_partials(nc, in_maps):
    res = run_bass_kernel_spmd(nc, in_maps, list(range(N_CORES)))
    tot = np.zeros(2, dtype=np.float32)
    for c in range(N_CORES):
        p = res.results[c]["partial"]
        tot += p.sum(axis=0, dtype=np.float32) if p.shape[0] > 1 else p[0]
    return np.asarray(np.float32(tot[0] / tot[1]))


def kernel(det_boxes, det_scores, det_labels, boxes, labels):
    det_boxes = np.asarray(det_boxes)
    det_labels = np.asarray(det_labels)
    boxes = np.asarray(boxes)
    labels = np.asarray(labels)
    in_maps = _prep_bucket_inputs(det_boxes, det_labels, boxes, labels)
    if in_maps is not None:
        return _run_partials(_get_bucket(), in_maps)
    in_maps = _prep_dense_inputs(det_boxes, det_labels, boxes, labels)
    return _run_partials(_get_dense(), in_maps)


# ---------------------------------------------------------------------------
# dev helpers (not used by the grading harness)

def _numpy_shard_ref(in_map):
    """Reference for one core's partial, straight from the sharded layout."""
    detp = in_map["detp"]  # [128, 5, T]
    objr = in_map["objr"]  # [5, F]
    det = detp.transpose(2, 0, 1).reshape(-1, 5)  # [DET_PAD, 5]
    dx1, dy1, dx2, dy2, dlab = det.T
    ox1, oy1, ox2, oy2, olab = objr
    ad = (dx2 - dx1) * (dy2 - dy1)
    ao = (ox2 - ox1) * (oy2 - oy1)
    wx = np.maximum(np.minimum(ox2[None], dx2[:, None])
                    - np.maximum(ox1[None], dx1[:, None]), 0)
    wy = np.maximum(np.minimum(oy2[None], dy2[:, None])
                    - np.maximum(oy1[None], dy1[:, None]), 0)
    inter = wx * wy
    denom = ao[None] + ad[:, None] - inter
    iou = inter / denom
    eq = (olab[None] == dlab[:, None]).astype(np.float32)
    miou = iou * eq
    best = miou.max(axis=0)
    hmv = eq.max(axis=0)
    return np.array([np.sum((1 - best) * hmv), np.sum(hmv)], dtype=np.float32)


def _full_numpy_ref(det_boxes, det_labels, boxes, labels):
    ov_all = []
    for c0 in range(0, N_OBJ, 256):
        b = boxes[c0:c0 + 256].astype(np.float64)
        d = det_boxes.astype(np.float64)
        lo = np.maximum(b[:, None, :2], d[None, :, :2])
        hi = np.minimum(b[:, None, 2:], d[None, :, 2:])
        wh = np.clip(hi - lo, 0, None)
        inter = wh[..., 0] * wh[..., 1]
        ao = (b[:, 2] - b[:, 0]) * (b[:, 3] - b[:, 1])
        ad = (d[:, 2] - d[:, 0]) * (d[:, 3] - d[:, 1])
        union = ao[:, None] + ad[None, :] - inter
        iou = inter / union
        same = labels[c0:c0 + 256, None] == det_labels[None, :]
        masked = np.where(same, iou, -np.inf)
        ov_all.append((masked.max(axis=1), same.any(axis=1)))
    best = np.concatenate([x[0] for x in ov_all])
    hmv = np.concatenate([x[1] for x in ov_all])
    npos = hmv.sum()
    return np.float32(np.sum(np.where(hmv, 1.0 - best, 0.0)) / npos)


def _rand_inputs(seed=0):
    rng = np.random.default_rng(seed)
    def mk(n):
        cxy = rng.random((n, 2), dtype=np.float32)
        wh = rng.random((n, 2), dtype=np.float32) * 0.3 + 0.02
        lo = np.clip(cxy - wh / 2, 0, 1)
        hi = np.clip(cxy + wh / 2, 0, 1)
        return np.concatenate([lo, hi], axis=1)
    return (mk(N_DET), rng.integers(0, 21, N_DET),
            mk(N_OBJ), rng.integers(0, 21, N_OBJ))


def _sim_core(nc, in_map, out_name="partial"):
    from concourse.bass_interp import CoreSim
    sim = CoreSim(nc)
    for k, v in in_map.items():
        sim.tensor(k)[:] = v
    sim.simulate()
    return np.array(sim.tensor(out_name))


def _selftest_sim():
    det_boxes, det_labels, boxes, labels = _rand_inputs(0)
    want_loss = _full_numpy_ref(det_boxes, det_labels, boxes, labels)

    # bucketed: simulate every core, combine
    in_maps = _prep_bucket_inputs(det_boxes, det_labels, boxes, labels)
    assert in_maps is not None
    nc = _get_bucket()
    tot = np.zeros(2, dtype=np.float32)
    for c in range(N_CORES):
        p = _sim_core(nc, in_maps[c])
        tot += p.sum(axis=0, dtype=np.float32) if p.shape[0] > 1 else p[0]
    got = np.float32(tot[0] / tot[1])
    print(f"bucket sim loss: {got}  numpy ref: {want_loss}  "
          f"relerr: {abs(got - want_loss) / abs(want_loss):.3e}")


if __name__ == "__main__":
    _selftest_sim()

